# revision 53
# baseline (speedup 1.0000x reference)
"""Trainium2 Bass kernel for a GQA attention block (B=2, S=2048, H=2048, 32 q-heads,
8 kv-heads, head_dim 64), tensor-parallel over heads across 8 NeuronCores.

v12 (from v2 at 583us -> ~438us causal). Changes vs v2:
  * RDH-shaped warm-up AllGather issued first: absorbs the CC-init barrier,
    cross-core launch skew AND the one-time RDH staging cost that previously
    stalled the whole pipeline ~65us mid-kernel on the first real AllGather.
  * q and kv projections merged per (batch, chunk): one x load instead of two
    (halves x HBM traffic); rope tables resident in SBUF for the whole kernel;
    weights host-packed into exact SBUF tile layouts so every weight DMA is a
    contiguous full-bandwidth copy (lead-in went 47us -> 16us to first MM).
  * projections and o_proj emitted as fine-grained filler generators (>=4
    matmuls per unit) driven between attention iterations, so the in-order PE
    queue has independent work between scores(j+1) and AV(j) instead of
    head-of-line blocking on the exp semaphore (v2: 247us of AV wait with the
    PE stuck at the cold 1.2GHz HAM clock).
  * ALL attention iterations emitted as one flat stream with a 1-unit scores
    lookahead crossing head-pair and chunk boundaries - the ACT exp pipeline
    never restarts.
  * causal mask applied AFTER exp as a cheap bf16 memset + one [128,2,128]
    binary-mask multiply on eS (replaces two [128,512] f32 PSUM adds per
    diagonal chunk that sat on the scores->exp critical chain).
  * last chunk's AllGather split per head-pair with an interleaved o_proj
    accumulation (p0 contraction chunks run while AG(p1) is in flight),
    shrinking the exposed tail.
Layout strategy unchanged: feature-major [dim, seq], scores^T with keys on
partitions and row-pair (base-partition 0/64) packing of two heads per score
matmul pair, AV via lhsT=V_nat with an appended ones column accumulating
softmax denominators for free, chunked AllGather + o_proj.
"""

import sys

if "/opt/trn_rl_repo" not in sys.path:
    sys.path.insert(0, "/opt/trn_rl_repo")

from collections import deque

import ml_dtypes
import numpy as np

import concourse.bass as bass
import concourse.mybir as mybir
import concourse.tile as tile
from concourse import bacc
from concourse.bass_utils import run_bass_kernel_spmd

B, S, H = 2, 2048, 2048
NH, NKV, HD = 32, 8, 64
NC = 8
HPC = NH // NC          # 4 q heads per core
QD = HPC * HD           # 256 q dims per core
NK = H // 128           # 16 contraction chunks
SC = 512                # seq chunk (matmul moving dim)
NQ = S // SC            # 4 seq chunks
NEG = np.float32(-1e30)
F32 = mybir.dt.float32
BF16 = mybir.dt.bfloat16

SWAP32 = [i ^ 1 for i in range(32)]   # stream_shuffle pair-swap mask


def build(mode, shared_rope=True):
    """mode: 'zeros' | 'causal' | 'general'."""
    MMDT = BF16
    PJDT = BF16
    DRDT = BF16

    nc = bacc.Bacc("TRN2", target_bir_lowering=False, debug=False, num_devices=NC)

    xT = nc.dram_tensor("xT", [B, H, S], DRDT, kind="ExternalInput").ap()
    # weights pre-packed on the host into the exact SBUF tile layouts so every
    # weight DMA is a full-bandwidth contiguous copy (v3 used 256B-descriptor
    # strided gathers that serialized the whole lead-in)
    qwP = nc.dram_tensor("qwP", [2, 2, 128, NK // 2, 128], DRDT, kind="ExternalInput").ap()
    kvwP = nc.dram_tensor("kvwP", [128, NK, 128], DRDT, kind="ExternalInput").ap()
    owP = nc.dram_tensor("owP", [128, NK, QD], BF16, kind="ExternalInput").ap()
    qb = nc.dram_tensor("qb", [QD, 1], F32, kind="ExternalInput").ap()
    qbs = nc.dram_tensor("qbs", [QD, 1], F32, kind="ExternalInput").ap()
    kvb = nc.dram_tensor("kvb", [128, 1], F32, kind="ExternalInput").ap()
    kvbs = nc.dram_tensor("kvbs", [128, 1], F32, kind="ExternalInput").ap()
    ropeC = nc.dram_tensor("ropeC", [B, 128, S], F32, kind="ExternalInput").ap()
    ropeS = nc.dram_tensor("ropeS", [B, 128, S], F32, kind="ExternalInput").ap()
    ident = nc.dram_tensor("ident", [64, 64], F32, kind="ExternalInput").ap()
    if mode == "causal":
        binm = nc.dram_tensor("binm", [128, 128], BF16, kind="ExternalInput").ap()
    if mode == "general":
        maskT = nc.dram_tensor("maskT", [S, S], F32, kind="ExternalInput").ap()

    y = nc.dram_tensor("y", [B * QD, S], F32, kind="ExternalOutput").ap()

    Exp = mybir.ActivationFunctionType.Exp
    Add = mybir.AluOpType.add
    Mult = mybir.AluOpType.mult

    with tile.TileContext(nc) as tc:
        with (
            tc.tile_pool(name="wpool", bufs=1) as wpool,
            tc.tile_pool(name="xin", bufs=3 if shared_rope else 2) as xpool,
            tc.tile_pool(name="acts", bufs=1) as apool,
            tc.tile_pool(name="tmp", bufs=2) as tpool,
            tc.tile_pool(name="exp", bufs=5) as epool,
            tc.tile_pool(name="norm", bufs=2) as npool,
            tc.tile_pool(name="attc", bufs=2) as atpool,
            tc.tile_pool(name="ost", bufs=2) as opool,
            tc.tile_pool(name="psS", bufs=2, space="PSUM") as psS,
            tc.tile_pool(name="psV", bufs=1, space="PSUM") as psV,
            tc.tile_pool(name="psP", bufs=2, space="PSUM") as psP,
            tc.tile_pool(name="dram", bufs=1, space="DRAM") as dpool,
            tc.tile_pool(name="mask", bufs=6 if mode == "general" else 1) as mpool,
        ):
            # ---- constants / weights; DMA order = first-use order so the
            # first kv-projection matmuls start as early as possible ----
            kvw_sb = wpool.tile([128, NK, 128], PJDT, name="kvw_sb")
            nc.sync.dma_start(kvw_sb[:], kvwP[:])

            def load_x_half(b, n, hf):
                # quarter-granularity DMAs: the first projection matmuls can
                # start as soon as the first 4 contraction chunks land, and
                # the transfers spread across more DMA queues
                xTb = xT[b].rearrange("(k p) s -> p k s", p=128)
                nsl = slice(n * SC, (n + 1) * SC)
                t = xpool.tile([128, NK // 2, SC], PJDT, name="xh", tag=f"xh{hf}")
                for q in range(2):
                    nc.sync.dma_start(
                        t[:, 4 * q : 4 * (q + 1), :],
                        xTb[:, hf * (NK // 2) + 4 * q : hf * (NK // 2) + 4 * (q + 1), nsl],
                    )
                return t

            def load_x_early(b, n):
                return [load_x_half(b, n, 0), load_x_half(b, n, 1)]

            # lead-in DMA interleave: x half -> q weights m0 -> x half -> m1,
            # so the q-projection weights arrive while the kv matmuls are
            # still consuming x (v17 queued all of x ahead of qw, leaving an
            # ~11us PE gap between the kv and q matmul groups)
            xh00 = [load_x_half(0, 0, 0)]
            qw_sb = []
            for m in range(2):
                halves = []
                for hf in range(2):
                    t = wpool.tile([128, NK // 2, 128], PJDT, name=f"qw{m}_{hf}")
                    nc.sync.dma_start(t[:], qwP[m, hf])
                    halves.append(t)
                qw_sb.append(halves)
                if m == 0:
                    xh00.append(load_x_half(0, 0, 1))

            # ---- RDH-shaped warm-up collective, emitted after the compute-
            # critical first loads: absorbs the CC init barrier, cross-core
            # launch skew and the one-time RDH staging while projections run
            warm_in = dpool.tile([QD, SC], MMDT, name="warm_in")
            warm_out = dpool.tile([H, SC], MMDT, name="warm_out", addr_space="Shared")
            nc.sync.dma_start(warm_in[:], xT[0][0:QD, 0:SC])
            nc.gpsimd.collective_compute(
                "AllGather",
                mybir.AluOpType.bypass,
                replica_groups=[list(range(NC))],
                ins=[warm_in.opt()],
                outs=[warm_out.opt()],
            )

            id_sb = wpool.tile([128, 64], F32, name="id_sb")
            nc.sync.dma_start(id_sb[64:128, :], ident[:])
            kvb_sb = wpool.tile([128, 1], F32, name="kvb_sb")
            nc.sync.dma_start(kvb_sb[:], kvb[:])
            kvbs_sb = wpool.tile([128, 1], F32, name="kvbs_sb")
            nc.sync.dma_start(kvbs_sb[:], kvbs[:])
            qb_sb, qbs_sb = [], []
            for m in range(2):
                t = wpool.tile([128, 1], F32, name=f"qb{m}")
                nc.sync.dma_start(t[:], qb[128 * m : 128 * (m + 1), :])
                qb_sb.append(t)
                t2 = wpool.tile([128, 1], F32, name=f"qbs{m}")
                nc.sync.dma_start(t2[:], qbs[128 * m : 128 * (m + 1), :])
                qbs_sb.append(t2)
            ones_sb = wpool.tile([128, NK], F32, name="ones_sb")
            nc.vector.memset(ones_sb[:], 1.0)

            # rope tables resident in SBUF for the whole kernel (after the
            # projection weights: only the DVE rope chain needs them)
            rope_sb = {}
            if shared_rope:
                rC = wpool.tile([128, S], F32, name="ropeC_sb")
                nc.sync.dma_start(rC[:], ropeC[0][:])
                rS = wpool.tile([128, S], F32, name="ropeS_sb")
                nc.sync.dma_start(rS[:], ropeS[0][:])
                rope_sb[0] = rope_sb[1] = (rC, rS)
            else:
                for b in range(B):
                    rC = wpool.tile([128, S], F32, name=f"ropeC_sb{b}")
                    nc.sync.dma_start(rC[:], ropeC[b][:])
                    rS = wpool.tile([128, S], F32, name=f"ropeS_sb{b}")
                    nc.sync.dma_start(rS[:], ropeS[b][:])
                    rope_sb[b] = (rC, rS)

            xh01 = load_x_early(0, 1)

            if mode == "causal":
                # binary lower-triangle mask for the one diagonal 128x128
                # block of each diagonal key-chunk, duplicated across the
                # head-pair axis; applied to eS AFTER exp so the scores->exp
                # chain never waits on the DVE
                binm_sb = wpool.tile([128, 2, 128], BF16, name="binm_sb")
                for seg in range(2):
                    nc.sync.dma_start(binm_sb[:, seg, :], binm[:])

            ow_sb = wpool.tile([128, NK, QD], MMDT, name="ow_sb")
            nc.sync.dma_start(ow_sb[:], owP[:])

            ag_out = {}
            for b in range(B):
                for n in range(NQ):
                    ag_out[(b, n)] = dpool.tile(
                        [H, SC], MMDT, name=f"ag_out{b}_{n}", addr_space="Shared"
                    )
            ag_out_sp = {}
            for p in range(2):
                ag_out_sp[(1, 3, p)] = dpool.tile(
                    [H // 2, SC], MMDT, name=f"ag_out_sp{p}", addr_space="Shared"
                )
            bt = {}

            def alloc_batch(b):
                qT = [
                    apool.tile([128, S], PJDT, name=f"qT{b}{p}", tag=f"qT{b}{p}")
                    for p in range(2)
                ]
                sk = apool.tile([128, S], PJDT, name=f"sk{b}", tag=f"sk{b}")
                vnat = apool.tile([128, NK * 65], MMDT, name=f"vnat{b}", tag=f"vnat{b}")
                nc.vector.tensor_copy(
                    vnat.rearrange("p (j c) -> p j c", c=65)[:, :, 64:65], ones_sb[:]
                )
                bt[b] = dict(qT=qT, sk=sk, vnat=vnat)

            load_x = load_x_early

            # ---- filler machinery: generators yielding small PE units ----
            fillers = deque()
            gens = {}

            def drive(k=1):
                done = 0
                while done < k and fillers:
                    try:
                        next(fillers[0])
                        done += 1
                    except StopIteration:
                        fillers.popleft()

            def run_now(g):
                for _ in g:
                    pass

            def force_done(key):
                # safety: fully emit a generator a dependent chunk needs NOW
                g = gens.pop(key, None)
                if g is not None:
                    run_now(g)

            def gen_proj(b, n, xh):
                """kv projection + q projection for chunk (b, n), one x load.

                Yield boundaries are chosen so EVERY unit carries >=4 PE
                matmuls (DVE rope work rides along inside MM-bearing units)
                to keep the in-order PE queue fed on every drive().
                """
                sk, vnat, qT = bt[b]["sk"], bt[b]["vnat"], bt[b]["qT"]
                rCt, rSt = rope_sb[b]
                nsl = slice(n * SC, (n + 1) * SC)
                st_ = 4 if b == 0 else 2
                # kv projection
                ps = psP.tile([128, SC], F32, name="ps_kv", tag="psP")
                for k in range(NK):
                    nc.tensor.matmul(
                        ps[:], kvw_sb[:, k, :], xh[k // 8][:, k % 8, :],
                        start=(k == 0), stop=(k == NK - 1),
                    )
                    if k % st_ == st_ - 1:
                        yield
                # unit: kv rope DVE chain + q(m=0) k0-3
                xs = tpool.tile([128, SC], F32, name="xs", tag="xs")
                nc.vector.stream_shuffle(xs[0:64, :], ps[0:64, :], SWAP32)
                m1 = tpool.tile([128, SC], F32, name="m1", tag="m1")
                nc.vector.scalar_tensor_tensor(
                    m1[0:64, :], ps[0:64, :], kvb_sb[0:64, :], rCt[0:64, nsl],
                    op0=Add, op1=Mult,
                )
                m2 = tpool.tile([128, SC], F32, name="m2", tag="m2")
                nc.vector.scalar_tensor_tensor(
                    m2[0:64, :], xs[0:64, :], kvbs_sb[0:64, :], rSt[0:64, nsl],
                    op0=Add, op1=Mult,
                )
                nc.vector.tensor_add(sk[0:64, nsl], m1[0:64, :], m2[0:64, :])
                nc.sync.dma_start(sk[64:128, nsl], sk[0:64, nsl])
                vst = tpool.tile([128, SC], F32, name="vst", tag="vst")
                nc.vector.tensor_scalar_add(
                    vst[64:128, :], ps[64:128, :], kvb_sb[64:128, :]
                )
                qps = [None, None]
                qps[0] = psP.tile([128, SC], F32, name="ps_q", tag="psP")
                for k in range(4):
                    nc.tensor.matmul(
                        qps[0][:], qw_sb[0][k // 8][:, k % 8, :], xh[k // 8][:, k % 8, :],
                        start=(k == 0), stop=False,
                    )
                yield
                # units: rest of q(m=0) with V transposes riding along
                for k in range(4, NK):
                    nc.tensor.matmul(
                        qps[0][:], qw_sb[0][k // 8][:, k % 8, :], xh[k // 8][:, k % 8, :],
                        start=False, stop=(k == NK - 1),
                    )
                    if k == 7 or k == 11:
                        js0 = 0 if k == 7 else 2
                        for js in (js0, js0 + 1):
                            j = 4 * n + js
                            tp = psP.tile([128, 64], F32, name="tp", tag="psP")
                            nc.tensor.transpose(
                                tp[:],
                                vst[64:128, 128 * js : 128 * (js + 1)],
                                id_sb[64:128, :],
                            )
                            nc.vector.tensor_copy(
                                vnat[:, j * 65 : j * 65 + 64], tp[:]
                            )
                    if k % st_ == st_ - 1 and k < NK - 1:
                        yield
                # unit: q(m0) rope + q(m1) k0-3
                def q_rope(m, ps_q):
                    xs = tpool.tile([128, SC], F32, name="xs", tag="xs")
                    nc.vector.stream_shuffle(xs[:], ps_q[:], SWAP32)
                    m1 = tpool.tile([128, SC], F32, name="m1", tag="m1")
                    nc.vector.scalar_tensor_tensor(
                        m1[:], ps_q[:], qb_sb[m][:], rCt[:, nsl], op0=Add, op1=Mult
                    )
                    m2 = tpool.tile([128, SC], F32, name="m2", tag="m2")
                    nc.vector.scalar_tensor_tensor(
                        m2[:], xs[:], qbs_sb[m][:], rSt[:, nsl], op0=Add, op1=Mult
                    )
                    nc.vector.tensor_add(qT[m][:, nsl], m1[:], m2[:])

                q_rope(0, qps[0])
                qps[1] = psP.tile([128, SC], F32, name="ps_q", tag="psP")
                for k in range(4):
                    nc.tensor.matmul(
                        qps[1][:], qw_sb[1][k // 8][:, k % 8, :], xh[k // 8][:, k % 8, :],
                        start=(k == 0), stop=False,
                    )
                yield
                for k in range(4, NK):
                    nc.tensor.matmul(
                        qps[1][:], qw_sb[1][k // 8][:, k % 8, :], xh[k // 8][:, k % 8, :],
                        start=False, stop=(k == NK - 1),
                    )
                    if k % st_ == st_ - 1 and k < NK - 1:
                        yield
                q_rope(1, qps[1])
                yield

            def push_proj(b, n):
                xh = load_x(b, n)          # DMA issued now, MMs ride the queue
                g = gen_proj(b, n, xh)
                gens[("proj", b, n)] = g
                fillers.append(g)

            def gen_oproj(b, n, agr):
                nsl = slice(n * SC, (n + 1) * SC)
                for mt_i in range(2):
                    ps = psP.tile([128, SC], F32, name="ps_o", tag="psP")
                    for k in range(NK):
                        nc.tensor.matmul(
                            ps[:],
                            ow_sb[:, k, 128 * mt_i : 128 * (mt_i + 1)],
                            agr[k // 8][:, k % 8, :],
                            start=(k == 0), stop=(k == NK - 1),
                        )
                        if k % 2 == 1 and k < NK - 1:
                            yield
                    st = opool.tile([128, SC], F32, name="st")
                    nc.vector.tensor_copy(st[:], ps[:])
                    nc.sync.dma_start(
                        y[b * QD + 128 * mt_i : b * QD + 128 * (mt_i + 1), nsl],
                        st[:],
                    )
                    yield

            def load_agr(b, n):
                agv = ag_out[(b, n)].rearrange("(k p) s -> p k s", p=128)
                agr = []
                for hf in range(2):
                    t = mpool.tile(
                        [128, NK // 2, SC], MMDT, name="agr", tag="agr", bufs=4
                    )
                    # quarter-granularity loads: first o_proj matmuls start as
                    # soon as the first 4 contraction chunks land
                    for q in range(2):
                        nc.sync.dma_start(
                            t[:, 4 * q : 4 * (q + 1), :],
                            agv[:, hf * (NK // 2) + 4 * q : hf * (NK // 2) + 4 * (q + 1), :],
                        )
                    agr.append(t)
                return agr

            agrs = {}

            def prefetch_oproj(b, n):
                # issue the agr DMAs well before the gen's MMs can reach the
                # head of the PE queue
                agrs[(b, n)] = load_agr(b, n)

            def push_oproj(b, n):
                agr = agrs.pop((b, n), None) or load_agr(b, n)
                g = gen_oproj(b, n, agr)
                gens[("oproj", b, n)] = g
                fillers.append(g)

            def emit_attn_stream(chunks):
                """All attention iterations of every (chunk, head-pair) as ONE
                flat stream with a 1-unit scores lookahead that crosses pair
                and chunk boundaries, so the ACT exp pipeline never restarts.

                chunks entries: dict(b, n, rate=1, pre=[...], acts={(p,j):
                [...]}, split_ag=False). `pre` callables run right before the
                chunk's first scores emission (emission-order safety for
                deps); `acts` run at the given (p, j) iteration.
                """
                seq = []
                for ci, ch in enumerate(chunks):
                    ch["jmax"] = (4 * ch["n"] + 4) if mode == "causal" else NK
                    for p in range(2):
                        for j in range(ch["jmax"]):
                            seq.append((ci, p, j))
                pS_tiles = {}
                pO = {}
                att = {}

                def emit_scores(u):
                    ci, p, j = u
                    ch = chunks[ci]
                    if p == 0 and j == 0:
                        for f in ch.get("pre", ()):
                            f()
                    b, n = ch["b"], ch["n"]
                    qT, sk = bt[b]["qT"], bt[b]["sk"]
                    nsl = slice(n * SC, (n + 1) * SC)
                    ksl = slice(j * 128, (j + 1) * 128)
                    pS = psS.tile([128, 2, SC], F32, name="pS", tag="psS")
                    nc.tensor.matmul(
                        pS[:, 0, :], sk[0:64, ksl], qT[p][0:64, nsl],
                        start=True, stop=True,
                    )
                    nc.tensor.matmul(
                        pS[:, 1, :], sk[64:128, ksl], qT[p][64:128, nsl],
                        start=True, stop=True,
                    )
                    pS_tiles[u] = pS

                def emit_exp_av(u):
                    ci, p, j = u
                    ch = chunks[ci]
                    b, n, jmax = ch["b"], ch["n"], ch["jmax"]
                    vnat = bt[b]["vnat"]
                    nsl = slice(n * SC, (n + 1) * SC)
                    pS = pS_tiles.pop(u)
                    eS = epool.tile([128, 2, SC], MMDT, name="eS", tag="eS")
                    if mode == "causal" and j >= 4 * n:
                        r = j - 4 * n
                        nc.scalar.activation(eS[:], pS[:], Exp, scale=0.125)
                        if r:
                            nc.vector.memset(eS[:, :, 0 : 128 * r], 0.0)
                        msl = slice(128 * r, 128 * (r + 1))
                        nc.vector.tensor_mul(
                            eS[:, :, msl], eS[:, :, msl], binm_sb[:]
                        )
                    elif mode == "general":
                        mt = mpool.tile([128, SC], F32, name="mt", tag="mt")
                        nc.sync.dma_start(
                            mt[:], maskT[128 * j : 128 * (j + 1), nsl]
                        )
                        nc.vector.scalar_tensor_tensor(
                            pS[:, 0, :], pS[:, 0, :], 0.125, mt[:], op0=Mult, op1=Add
                        )
                        nc.vector.scalar_tensor_tensor(
                            pS[:, 1, :], pS[:, 1, :], 0.125, mt[:], op0=Mult, op1=Add
                        )
                        nc.scalar.activation(eS[:], pS[:], Exp, scale=1.0)
                    else:
                        nc.scalar.activation(eS[:], pS[:], Exp, scale=0.125)
                    if j == 0:
                        pO[(ci, p)] = psV.tile([65, 2, SC], F32, name="pO", tag="psV")
                    po = pO[(ci, p)]
                    vsl = slice(j * 65, (j + 1) * 65)
                    nc.tensor.matmul(
                        po[:, 0, :], vnat[:, vsl], eS[:, 0, :],
                        start=(j == 0), stop=(j == jmax - 1),
                    )
                    nc.tensor.matmul(
                        po[:, 1, :], vnat[:, vsl], eS[:, 1, :],
                        start=(j == 0), stop=(j == jmax - 1),
                    )

                def emit_tail(u):
                    ci, p, j = u
                    ch = chunks[ci]
                    if j != ch["jmax"] - 1:
                        return
                    b, n = ch["b"], ch["n"]
                    po = pO.pop((ci, p))
                    last_unit = ci == len(chunks) - 1 and p == 1
                    pOc = npool.tile([65, 2, SC], F32, name="pOc", tag="pOc")
                    den0 = npool.tile([1, 2, SC], F32, name="den0", tag="den0", bufs=1)
                    if last_unit:
                        # tail-critical: pull the denominator row out first so
                        # recip + broadcast overlap the bulk PSUM drain
                        nc.vector.tensor_copy(den0[:], po[64:65, :, :])
                        nc.vector.reciprocal_approx_fast(den0[:], den0[:])
                        nc.vector.tensor_copy(pOc[:], po[:])
                        drive(1)
                    else:
                        # drain psum fast (frees the bank), then normalize
                        nc.vector.tensor_copy(pOc[:], po[:])
                        drive(1)
                        nc.sync.dma_start(den0[:], pOc[64:65, :, :])
                        nc.vector.reciprocal_approx_fast(den0[:], den0[:])
                    rb = npool.tile([64, 2, SC], F32, name="rb", tag="rb", bufs=1)
                    nc.gpsimd.partition_broadcast(rb[:], den0[:])
                    aT = atpool.tile([128, SC], MMDT, name=f"aT{p}", tag=f"aT{p}")
                    nc.vector.tensor_mul(aT[0:64, :], pOc[0:64, 0, :], rb[:, 0, :])
                    tb = npool.tile([64, SC], MMDT, name="tb", tag="tb", bufs=2)
                    nc.vector.tensor_mul(tb[:], pOc[0:64, 1, :], rb[:, 1, :])
                    nc.sync.dma_start(aT[64:128, :], tb[:])
                    att[(ci, p)] = aT
                    if ch.get("split_ag"):
                        # per-head-pair AllGather: p0's gather overlaps p1's
                        # attention so only p1's half is exposed in the tail
                        agi = dpool.tile(
                            [128, SC], MMDT, name=f"agis{b}_{n}_{p}",
                            tag="ag_in", bufs=4,
                        )
                        nc.sync.dma_start(agi[:], aT[:])
                        nc.gpsimd.collective_compute(
                            "AllGather",
                            mybir.AluOpType.bypass,
                            replica_groups=[list(range(NC))],
                            ins=[agi.opt()],
                            outs=[ag_out_sp[(b, n, p)].opt()],
                        )
                        drive(1)
                    elif p == 1:
                        ag_in = dpool.tile(
                            [QD, SC], MMDT, name=f"ag_in{b}_{n}", tag="ag_in", bufs=4
                        )
                        for pp in range(2):
                            nc.sync.dma_start(
                                ag_in[128 * pp : 128 * (pp + 1), :], att[(ci, pp)][:]
                            )
                        nc.gpsimd.collective_compute(
                            "AllGather",
                            mybir.AluOpType.bypass,
                            replica_groups=[list(range(NC))],
                            ins=[ag_in.opt()],
                            outs=[ag_out[(b, n)].opt()],
                        )
                        drive(2)

                emit_scores(seq[0])
                for i, u in enumerate(seq):
                    ci, p, j = u
                    ch = chunks[ci]
                    for f in ch.get("acts", {}).get((p, j), ()):
                        f()
                    if i + 1 < len(seq):
                        emit_scores(seq[i + 1])
                    if j % ch.get("rate", 1) == 0:
                        drive(1)
                    emit_exp_av(u)
                    emit_tail(u)

            def load_agr_split(b, n, p01):
                agv = ag_out_sp[(b, n, p01)].rearrange("(c r) s -> r c s", r=128)
                t = mpool.tile([128, NK // 2, SC], MMDT, name="agrs", tag="agr", bufs=4)
                for q in range(2):
                    nc.sync.dma_start(
                        t[:, 4 * q : 4 * (q + 1), :], agv[:, 4 * q : 4 * (q + 1), :]
                    )
                return t

            def emit_oproj_split(b, n):
                # o_proj consuming the two per-head-pair AllGathers: the p0
                # contraction chunks run while AG(p1) is still in flight
                nsl = slice(n * SC, (n + 1) * SC)
                ow_v = ow_sb.rearrange("p (c two) q -> p two c q", two=2)
                agr0 = agrs.pop((b, n, 0), None) or load_agr_split(b, n, 0)
                agr1 = load_agr_split(b, n, 1)
                ps = []
                for mt_i in range(2):
                    ps.append(psP.tile([128, SC], F32, name="ps_o", tag="psP"))
                    for c in range(8):
                        nc.tensor.matmul(
                            ps[mt_i][:],
                            ow_v[:, 0, c, 128 * mt_i : 128 * (mt_i + 1)],
                            agr0[:, c, :],
                            start=(c == 0), stop=False,
                        )
                for mt_i in range(2):
                    for c in range(8):
                        nc.tensor.matmul(
                            ps[mt_i][:],
                            ow_v[:, 1, c, 128 * mt_i : 128 * (mt_i + 1)],
                            agr1[:, c, :],
                            start=False, stop=(c == 7),
                        )
                    st = opool.tile([128, SC], F32, name="st")
                    nc.vector.tensor_copy(st[:], ps[mt_i][:])
                    nc.sync.dma_start(
                        y[b * QD + 128 * mt_i : b * QD + 128 * (mt_i + 1), nsl],
                        st[:],
                    )

            # ---- schedule ----
            # lead-in: proj(0,0) and proj(0,1) run dense on the PE while the
            # warm-up AllGather absorbs the collective start-up costs; the
            # remaining projections and all o_proj chunks ride the filler
            # queue between attention iterations.
            alloc_batch(0)
            alloc_batch(1)
            run_now(gen_proj(0, 0, xh00))
            run_now(gen_proj(0, 1, xh01))
            push_proj(0, 2)
            push_proj(0, 3)
            # in causal mode attention chunk (b,n) only reads keys j<=4n+3,
            # so later projections can ride the filler queue; in zeros/general
            # mode EVERY chunk reads the full key range, so all of a batch's
            # kv projections must be fully emitted before its first chunk
            causal = mode == "causal"
            pre00 = [] if causal else [
                lambda: force_done(("proj", 0, 2)),
                lambda: force_done(("proj", 0, 3)),
            ]
            pre10 = [lambda: force_done(("proj", 1, 0))] if causal else [
                lambda: force_done(("proj", 1, 0)),
                lambda: force_done(("proj", 1, 1)),
                lambda: force_done(("proj", 1, 2)),
                lambda: force_done(("proj", 1, 3)),
            ]
            chunks = [
                dict(b=0, n=0, pre=pre00),
                dict(b=0, n=1, pre=[lambda: push_proj(1, 0)]),
                dict(b=0, n=2, pre=[lambda: push_proj(1, 1)]),
                dict(b=0, n=3, pre=[lambda: push_proj(1, 2),
                                    lambda: push_proj(1, 3),
                                    lambda: push_oproj(0, 0)]),
                dict(b=1, n=0,
                     pre=pre10 + [lambda: push_oproj(0, 1)]),
                dict(b=1, n=1,
                     pre=([lambda: force_done(("proj", 1, 1))] if causal else [])
                         + [lambda: push_oproj(0, 2)]),
                dict(b=1, n=2,
                     pre=([lambda: force_done(("proj", 1, 2))] if causal else [])
                         + [lambda: push_oproj(0, 3)]),
                dict(b=1, n=3, split_ag=True,
                     pre=([lambda: force_done(("proj", 1, 3))] if causal else [])
                         + [lambda: push_oproj(1, 0)],
                     acts={(0, 0): [lambda: push_oproj(1, 1)],
                           (0, 8): [lambda: push_oproj(1, 2)],
                           (1, 2): [lambda: agrs.__setitem__(
                               (1, 3, 0), load_agr_split(1, 3, 0))]}),
            ]
            emit_attn_stream(chunks)
            # drain whatever filler is left, then the tail o_proj
            while fillers:
                drive(1)
            emit_oproj_split(1, 3)
    nc.compile()
    return nc


_cache = {}


def _get_nc(mode, shared_rope):
    key = (mode, shared_rope)
    if key not in _cache:
        _cache[key] = build(mode, shared_rope)
    return _cache[key]


def _mode_of(mask):
    m = np.asarray(mask)
    if not np.any(m):
        return "zeros"
    m2 = m.reshape(S, S)
    tril = np.tril(np.ones((S, S), dtype=bool))
    if np.all(m2[tril] == 0.0) and np.all(m2[~tril] <= -1e30):
        return "causal"
    return "general"


def make_inputs(hidden_states, cos, sin, positions, mask, q_w, q_b, k_w, k_b,
                v_w, v_b, o_w, mode):
    """Host-side preprocessing -> list of per-core input dicts."""
    ddt = ml_dtypes.bfloat16
    hs = np.ascontiguousarray(np.asarray(hidden_states, dtype=np.float32))
    xT = np.ascontiguousarray(hs.transpose(0, 2, 1).astype(ddt))   # [B, H, S]
    cos = np.asarray(cos, dtype=np.float32)
    sin = np.asarray(sin, dtype=np.float32)
    pos = np.asarray(positions)
    cosg = cos[pos]                                            # [B, S, 32]
    sing = sin[pos]
    d = np.arange(64)
    idx = d % 32
    sign = np.where(d % 2 == 0, -1.0, 1.0).astype(np.float32)
    C64 = cosg[:, :, idx].transpose(0, 2, 1)                   # [B, 64, S]
    Sn64 = (sing[:, :, idx] * sign[None, None, :]).transpose(0, 2, 1)
    ropeC = np.ascontiguousarray(np.concatenate([C64, C64], axis=1))   # [B,128,S]
    ropeS = np.ascontiguousarray(np.concatenate([Sn64, Sn64], axis=1))
    ident = np.eye(64, dtype=np.float32)

    q_w = np.asarray(q_w, dtype=np.float32)
    k_w = np.asarray(k_w, dtype=np.float32)
    v_w = np.asarray(v_w, dtype=np.float32)
    o_w = np.asarray(o_w, dtype=np.float32)
    q_b = np.asarray(q_b, dtype=np.float32)
    k_b = np.asarray(k_b, dtype=np.float32)
    v_b = np.asarray(v_b, dtype=np.float32)

    extra = {}
    if mode == "causal":
        kk = np.arange(128)[:, None]
        qq = np.arange(128)[None, :]
        extra["binm"] = np.ascontiguousarray(
            np.where(kk <= qq, 1.0, 0.0).astype(ddt)
        )
    if mode == "general":
        extra["maskT"] = np.ascontiguousarray(
            np.asarray(mask, dtype=np.float32).reshape(S, S).T
        )

    in_maps = []
    for c in range(NC):
        qsl = slice(c * QD, (c + 1) * QD)
        ksl = slice(c * HD, (c + 1) * HD)
        qbc = q_b[qsl]
        kvb_c = np.concatenate([k_b[ksl], v_b[ksl]])
        # pack weights into the exact SBUF tile layouts (contiguous DMA)
        qwt = q_w[qsl].T.reshape(NK, 128, QD)           # [k, p, q]
        qwP = np.ascontiguousarray(
            np.stack([
                np.stack([
                    qwt[hf * 8 : (hf + 1) * 8, :, 128 * m : 128 * (m + 1)]
                    .transpose(1, 0, 2) for hf in range(2)
                ]) for m in range(2)
            ]).astype(ddt)
        )                                               # [2, 2, 128, 8, 128]
        kvw = np.concatenate([k_w[ksl], v_w[ksl]], axis=0).T.reshape(NK, 128, 128)
        kvwP = np.ascontiguousarray(kvw.transpose(1, 0, 2).astype(ddt))
        ow = o_w[qsl, :].T.reshape(NK, 128, QD)
        owP = np.ascontiguousarray(ow.transpose(1, 0, 2).astype(ddt))
        m = {
            "xT": xT,
            "qwP": qwP,
            "kvwP": kvwP,
            "owP": owP,
            "qb": np.ascontiguousarray(qbc[:, None]),
            "qbs": np.ascontiguousarray(qbc[np.arange(QD) ^ 1][:, None]),
            "kvb": np.ascontiguousarray(kvb_c[:, None]),
            "kvbs": np.ascontiguousarray(kvb_c[np.arange(128) ^ 1][:, None]),
            "ropeC": ropeC,
            "ropeS": ropeS,
            "ident": ident,
        }
        m.update(extra)
        in_maps.append(m)
    return in_maps


def assemble_output(shards):
    """shards: list of per-core y arrays [B*QD, S] -> [B, S, H] float32."""
    full = np.empty((B, H, S), dtype=np.float32)
    for c in range(NC):
        sh = shards[c].reshape(B, QD, S)
        for b in range(B):
            full[b, QD * c : QD * (c + 1)] = sh[b]
    return np.ascontiguousarray(full.transpose(0, 2, 1).astype(np.float32))


def kernel(**inputs):
    mode = _mode_of(inputs["mask"])
    pos = np.asarray(inputs["positions"])
    shared_rope = bool(np.array_equal(pos[0], pos[1]))
    nc = _get_nc(mode, shared_rope)
    in_maps = make_inputs(mode=mode, **{k: inputs[k] for k in (
        "hidden_states", "cos", "sin", "positions", "mask",
        "q_w", "q_b", "k_w", "k_b", "v_w", "v_b", "o_w")})
    res = run_bass_kernel_spmd(nc, in_maps, list(range(NC)))
    return assemble_output([res.results[c]["y"] for c in range(NC)])


# revision 54
# speedup vs baseline: 1.0736x; 1.0736x over previous
"""Trainium2 Bass kernel for a GQA attention block (B=2, S=2048, H=2048, 32 q-heads,
8 kv-heads, head_dim 64), tensor-parallel over heads across 8 NeuronCores.

v12 (from v2 at 583us -> ~438us causal). Changes vs v2:
  * RDH-shaped warm-up AllGather issued first: absorbs the CC-init barrier,
    cross-core launch skew AND the one-time RDH staging cost that previously
    stalled the whole pipeline ~65us mid-kernel on the first real AllGather.
  * q and kv projections merged per (batch, chunk): one x load instead of two
    (halves x HBM traffic); rope tables resident in SBUF for the whole kernel;
    weights host-packed into exact SBUF tile layouts so every weight DMA is a
    contiguous full-bandwidth copy (lead-in went 47us -> 16us to first MM).
  * projections and o_proj emitted as fine-grained filler generators (>=4
    matmuls per unit) driven between attention iterations, so the in-order PE
    queue has independent work between scores(j+1) and AV(j) instead of
    head-of-line blocking on the exp semaphore (v2: 247us of AV wait with the
    PE stuck at the cold 1.2GHz HAM clock).
  * ALL attention iterations emitted as one flat stream with a 1-unit scores
    lookahead crossing head-pair and chunk boundaries - the ACT exp pipeline
    never restarts.
  * causal mask applied AFTER exp as a cheap bf16 memset + one [128,2,128]
    binary-mask multiply on eS (replaces two [128,512] f32 PSUM adds per
    diagonal chunk that sat on the scores->exp critical chain).
  * last chunk's AllGather split per head-pair with an interleaved o_proj
    accumulation (p0 contraction chunks run while AG(p1) is in flight),
    shrinking the exposed tail.
Layout strategy unchanged: feature-major [dim, seq], scores^T with keys on
partitions and row-pair (base-partition 0/64) packing of two heads per score
matmul pair, AV via lhsT=V_nat with an appended ones column accumulating
softmax denominators for free, chunked AllGather + o_proj.
"""

import sys

if "/opt/trn_rl_repo" not in sys.path:
    sys.path.insert(0, "/opt/trn_rl_repo")

from collections import deque

import ml_dtypes
import numpy as np

import concourse.bass as bass
import concourse.mybir as mybir
import concourse.tile as tile
from concourse import bacc
from concourse.bass_utils import run_bass_kernel_spmd

B, S, H = 2, 2048, 2048
NH, NKV, HD = 32, 8, 64
NC = 8
HPC = NH // NC          # 4 q heads per core
QD = HPC * HD           # 256 q dims per core
NK = H // 128           # 16 contraction chunks
SC = 512                # seq chunk (matmul moving dim)
NQ = S // SC            # 4 seq chunks
NEG = np.float32(-1e30)
F32 = mybir.dt.float32
BF16 = mybir.dt.bfloat16

SWAP32 = [i ^ 1 for i in range(32)]   # stream_shuffle pair-swap mask


def build(mode, shared_rope=True):
    """mode: 'zeros' | 'causal' | 'general'."""
    MMDT = BF16
    PJDT = BF16
    DRDT = BF16

    nc = bacc.Bacc("TRN2", target_bir_lowering=False, debug=False, num_devices=NC)

    xT = nc.dram_tensor("xT", [B, H, S], DRDT, kind="ExternalInput").ap()
    # weights pre-packed on the host into the exact SBUF tile layouts so every
    # weight DMA is a full-bandwidth contiguous copy (v3 used 256B-descriptor
    # strided gathers that serialized the whole lead-in)
    qwP = nc.dram_tensor("qwP", [2, 2, 128, NK // 2, 128], DRDT, kind="ExternalInput").ap()
    kvwP = nc.dram_tensor("kvwP", [128, NK, 128], DRDT, kind="ExternalInput").ap()
    owP = nc.dram_tensor("owP", [128, NK, QD], BF16, kind="ExternalInput").ap()
    qb = nc.dram_tensor("qb", [QD, 1], F32, kind="ExternalInput").ap()
    qbs = nc.dram_tensor("qbs", [QD, 1], F32, kind="ExternalInput").ap()
    kvb = nc.dram_tensor("kvb", [128, 1], F32, kind="ExternalInput").ap()
    kvbs = nc.dram_tensor("kvbs", [128, 1], F32, kind="ExternalInput").ap()
    ropeC = nc.dram_tensor("ropeC", [B, 128, S], F32, kind="ExternalInput").ap()
    ropeS = nc.dram_tensor("ropeS", [B, 128, S], F32, kind="ExternalInput").ap()
    ident = nc.dram_tensor("ident", [64, 64], F32, kind="ExternalInput").ap()
    if mode == "causal":
        binm = nc.dram_tensor("binm", [128, 128], BF16, kind="ExternalInput").ap()
    if mode == "general":
        maskT = nc.dram_tensor("maskT", [S, S], F32, kind="ExternalInput").ap()

    y = nc.dram_tensor("y", [B * QD, S], F32, kind="ExternalOutput").ap()

    Exp = mybir.ActivationFunctionType.Exp
    Add = mybir.AluOpType.add
    Mult = mybir.AluOpType.mult

    with tile.TileContext(nc) as tc:
        with (
            tc.tile_pool(name="wpool", bufs=1) as wpool,
            tc.tile_pool(name="xin", bufs=3 if shared_rope else 2) as xpool,
            tc.tile_pool(name="acts", bufs=1) as apool,
            tc.tile_pool(name="tmp", bufs=2) as tpool,
            tc.tile_pool(name="exp", bufs=5) as epool,
            tc.tile_pool(name="norm", bufs=2) as npool,
            tc.tile_pool(name="attc", bufs=2) as atpool,
            tc.tile_pool(name="ost", bufs=2) as opool,
            tc.tile_pool(name="psS", bufs=2, space="PSUM") as psS,
            tc.tile_pool(name="psV", bufs=1, space="PSUM") as psV,
            tc.tile_pool(name="psP", bufs=2, space="PSUM") as psP,
            tc.tile_pool(name="dram", bufs=1, space="DRAM") as dpool,
            tc.tile_pool(name="mask", bufs=6 if mode == "general" else 1) as mpool,
        ):
            # ---- constants / weights; DMA order = first-use order so the
            # first kv-projection matmuls start as early as possible ----
            kvw_sb = wpool.tile([128, NK, 128], PJDT, name="kvw_sb")
            nc.sync.dma_start(kvw_sb[:], kvwP[:])

            def load_x_early(b, n):
                # quarter-granularity DMAs: the first projection matmuls can
                # start as soon as the first 4 contraction chunks land, and
                # the transfers spread across more DMA queues
                xTb = xT[b].rearrange("(k p) s -> p k s", p=128)
                nsl = slice(n * SC, (n + 1) * SC)
                xh = []
                for hf in range(2):
                    t = xpool.tile([128, NK // 2, SC], PJDT, name="xh", tag=f"xh{hf}")
                    for q in range(2):
                        nc.sync.dma_start(
                            t[:, 4 * q : 4 * (q + 1), :],
                            xTb[:, hf * (NK // 2) + 4 * q : hf * (NK // 2) + 4 * (q + 1), nsl],
                        )
                    xh.append(t)
                return xh

            xh00 = load_x_early(0, 0)

            # ---- RDH-shaped warm-up collective, emitted after the compute-
            # critical first loads: absorbs the CC init barrier, cross-core
            # launch skew and the one-time RDH staging while projections run
            warm_in = dpool.tile([QD, SC], MMDT, name="warm_in")
            warm_out = dpool.tile([H, SC], MMDT, name="warm_out", addr_space="Shared")
            nc.sync.dma_start(warm_in[:], xT[0][0:QD, 0:SC])
            nc.gpsimd.collective_compute(
                "AllGather",
                mybir.AluOpType.bypass,
                replica_groups=[list(range(NC))],
                ins=[warm_in.opt()],
                outs=[warm_out.opt()],
            )

            id_sb = wpool.tile([128, 64], F32, name="id_sb")
            nc.sync.dma_start(id_sb[64:128, :], ident[:])
            kvb_sb = wpool.tile([128, 1], F32, name="kvb_sb")
            nc.sync.dma_start(kvb_sb[:], kvb[:])
            kvbs_sb = wpool.tile([128, 1], F32, name="kvbs_sb")
            nc.sync.dma_start(kvbs_sb[:], kvbs[:])

            qw_sb = []
            for m in range(2):
                halves = []
                for hf in range(2):
                    t = wpool.tile([128, NK // 2, 128], PJDT, name=f"qw{m}_{hf}")
                    nc.sync.dma_start(t[:], qwP[m, hf])
                    halves.append(t)
                qw_sb.append(halves)
            qb_sb, qbs_sb = [], []
            for m in range(2):
                t = wpool.tile([128, 1], F32, name=f"qb{m}")
                nc.sync.dma_start(t[:], qb[128 * m : 128 * (m + 1), :])
                qb_sb.append(t)
                t2 = wpool.tile([128, 1], F32, name=f"qbs{m}")
                nc.sync.dma_start(t2[:], qbs[128 * m : 128 * (m + 1), :])
                qbs_sb.append(t2)
            ones_sb = wpool.tile([128, NK], F32, name="ones_sb")
            nc.vector.memset(ones_sb[:], 1.0)

            # rope tables resident in SBUF for the whole kernel (after the
            # projection weights: only the DVE rope chain needs them)
            rope_sb = {}
            if shared_rope:
                rC = wpool.tile([128, S], F32, name="ropeC_sb")
                nc.sync.dma_start(rC[:], ropeC[0][:])
                rS = wpool.tile([128, S], F32, name="ropeS_sb")
                nc.sync.dma_start(rS[:], ropeS[0][:])
                rope_sb[0] = rope_sb[1] = (rC, rS)
            else:
                for b in range(B):
                    rC = wpool.tile([128, S], F32, name=f"ropeC_sb{b}")
                    nc.sync.dma_start(rC[:], ropeC[b][:])
                    rS = wpool.tile([128, S], F32, name=f"ropeS_sb{b}")
                    nc.sync.dma_start(rS[:], ropeS[b][:])
                    rope_sb[b] = (rC, rS)

            xh01 = load_x_early(0, 1)

            if mode == "causal":
                # binary lower-triangle mask for the one diagonal 128x128
                # block of each diagonal key-chunk, duplicated across the
                # head-pair axis; applied to eS AFTER exp so the scores->exp
                # chain never waits on the DVE
                binm_sb = wpool.tile([128, 2, 128], BF16, name="binm_sb")
                for seg in range(2):
                    nc.sync.dma_start(binm_sb[:, seg, :], binm[:])

            ow_sb = wpool.tile([128, NK, QD], MMDT, name="ow_sb")
            nc.sync.dma_start(ow_sb[:], owP[:])

            ag_out = {}
            for b in range(B):
                for n in range(NQ):
                    ag_out[(b, n)] = dpool.tile(
                        [H, SC], MMDT, name=f"ag_out{b}_{n}", addr_space="Shared"
                    )
            ag_out_sp = {}
            for p in range(2):
                ag_out_sp[(1, 3, p)] = dpool.tile(
                    [H // 2, SC], MMDT, name=f"ag_out_sp{p}", addr_space="Shared"
                )
            bt = {}

            def alloc_batch(b):
                qT = [
                    apool.tile([128, S], PJDT, name=f"qT{b}{p}", tag=f"qT{b}{p}")
                    for p in range(2)
                ]
                sk = apool.tile([128, S], PJDT, name=f"sk{b}", tag=f"sk{b}")
                vnat = apool.tile([128, NK * 65], MMDT, name=f"vnat{b}", tag=f"vnat{b}")
                nc.vector.tensor_copy(
                    vnat.rearrange("p (j c) -> p j c", c=65)[:, :, 64:65], ones_sb[:]
                )
                bt[b] = dict(qT=qT, sk=sk, vnat=vnat)

            load_x = load_x_early

            # ---- filler machinery: generators yielding small PE units ----
            fillers = deque()
            gens = {}

            def drive(k=1):
                done = 0
                while done < k and fillers:
                    try:
                        next(fillers[0])
                        done += 1
                    except StopIteration:
                        fillers.popleft()

            def run_now(g):
                for _ in g:
                    pass

            def force_done(key):
                # safety: fully emit a generator a dependent chunk needs NOW
                g = gens.pop(key, None)
                if g is not None:
                    run_now(g)

            def gen_proj(b, n, xh):
                """kv projection + q projection for chunk (b, n), one x load.

                Yield boundaries are chosen so EVERY unit carries >=4 PE
                matmuls (DVE rope work rides along inside MM-bearing units)
                to keep the in-order PE queue fed on every drive().
                """
                sk, vnat, qT = bt[b]["sk"], bt[b]["vnat"], bt[b]["qT"]
                rCt, rSt = rope_sb[b]
                nsl = slice(n * SC, (n + 1) * SC)
                st_ = 4 if b == 0 else 2
                # kv projection
                ps = psP.tile([128, SC], F32, name="ps_kv", tag="psP")
                for k in range(NK):
                    nc.tensor.matmul(
                        ps[:], kvw_sb[:, k, :], xh[k // 8][:, k % 8, :],
                        start=(k == 0), stop=(k == NK - 1),
                    )
                    if k % st_ == st_ - 1:
                        yield
                # unit: kv rope DVE chain + q(m=0) k0-3
                xs = tpool.tile([128, SC], F32, name="xs", tag="xs")
                nc.vector.stream_shuffle(xs[0:64, :], ps[0:64, :], SWAP32)
                m1 = tpool.tile([128, SC], F32, name="m1", tag="m1")
                nc.vector.scalar_tensor_tensor(
                    m1[0:64, :], ps[0:64, :], kvb_sb[0:64, :], rCt[0:64, nsl],
                    op0=Add, op1=Mult,
                )
                m2 = tpool.tile([128, SC], F32, name="m2", tag="m2")
                nc.vector.scalar_tensor_tensor(
                    m2[0:64, :], xs[0:64, :], kvbs_sb[0:64, :], rSt[0:64, nsl],
                    op0=Add, op1=Mult,
                )
                nc.vector.tensor_add(sk[0:64, nsl], m1[0:64, :], m2[0:64, :])
                nc.sync.dma_start(sk[64:128, nsl], sk[0:64, nsl])
                vst = tpool.tile([128, SC], F32, name="vst", tag="vst")
                nc.vector.tensor_scalar_add(
                    vst[64:128, :], ps[64:128, :], kvb_sb[64:128, :]
                )
                qps = [None, None]
                qps[0] = psP.tile([128, SC], F32, name="ps_q", tag="psP")
                for k in range(4):
                    nc.tensor.matmul(
                        qps[0][:], qw_sb[0][k // 8][:, k % 8, :], xh[k // 8][:, k % 8, :],
                        start=(k == 0), stop=False,
                    )
                yield
                # units: rest of q(m=0) with V transposes riding along
                for k in range(4, NK):
                    nc.tensor.matmul(
                        qps[0][:], qw_sb[0][k // 8][:, k % 8, :], xh[k // 8][:, k % 8, :],
                        start=False, stop=(k == NK - 1),
                    )
                    if k == 7 or k == 11:
                        js0 = 0 if k == 7 else 2
                        for js in (js0, js0 + 1):
                            j = 4 * n + js
                            tp = psP.tile([128, 64], F32, name="tp", tag="psP")
                            nc.tensor.transpose(
                                tp[:],
                                vst[64:128, 128 * js : 128 * (js + 1)],
                                id_sb[64:128, :],
                            )
                            nc.vector.tensor_copy(
                                vnat[:, j * 65 : j * 65 + 64], tp[:]
                            )
                    if k % st_ == st_ - 1 and k < NK - 1:
                        yield
                # unit: q(m0) rope + q(m1) k0-3
                def q_rope(m, ps_q):
                    xs = tpool.tile([128, SC], F32, name="xs", tag="xs")
                    nc.vector.stream_shuffle(xs[:], ps_q[:], SWAP32)
                    m1 = tpool.tile([128, SC], F32, name="m1", tag="m1")
                    nc.vector.scalar_tensor_tensor(
                        m1[:], ps_q[:], qb_sb[m][:], rCt[:, nsl], op0=Add, op1=Mult
                    )
                    m2 = tpool.tile([128, SC], F32, name="m2", tag="m2")
                    nc.vector.scalar_tensor_tensor(
                        m2[:], xs[:], qbs_sb[m][:], rSt[:, nsl], op0=Add, op1=Mult
                    )
                    nc.vector.tensor_add(qT[m][:, nsl], m1[:], m2[:])

                q_rope(0, qps[0])
                qps[1] = psP.tile([128, SC], F32, name="ps_q", tag="psP")
                for k in range(4):
                    nc.tensor.matmul(
                        qps[1][:], qw_sb[1][k // 8][:, k % 8, :], xh[k // 8][:, k % 8, :],
                        start=(k == 0), stop=False,
                    )
                yield
                for k in range(4, NK):
                    nc.tensor.matmul(
                        qps[1][:], qw_sb[1][k // 8][:, k % 8, :], xh[k // 8][:, k % 8, :],
                        start=False, stop=(k == NK - 1),
                    )
                    if k % st_ == st_ - 1 and k < NK - 1:
                        yield
                q_rope(1, qps[1])
                yield

            def push_proj(b, n):
                xh = load_x(b, n)          # DMA issued now, MMs ride the queue
                g = gen_proj(b, n, xh)
                gens[("proj", b, n)] = g
                fillers.append(g)

            def gen_oproj(b, n, agr):
                nsl = slice(n * SC, (n + 1) * SC)
                for mt_i in range(2):
                    ps = psP.tile([128, SC], F32, name="ps_o", tag="psP")
                    for k in range(NK):
                        nc.tensor.matmul(
                            ps[:],
                            ow_sb[:, k, 128 * mt_i : 128 * (mt_i + 1)],
                            agr[k // 8][:, k % 8, :],
                            start=(k == 0), stop=(k == NK - 1),
                        )
                        if k % 2 == 1 and k < NK - 1:
                            yield
                    st = opool.tile([128, SC], F32, name="st")
                    nc.vector.tensor_copy(st[:], ps[:])
                    nc.sync.dma_start(
                        y[b * QD + 128 * mt_i : b * QD + 128 * (mt_i + 1), nsl],
                        st[:],
                    )
                    yield

            def load_agr(b, n):
                agv = ag_out[(b, n)].rearrange("(k p) s -> p k s", p=128)
                agr = []
                for hf in range(2):
                    t = mpool.tile(
                        [128, NK // 2, SC], MMDT, name="agr", tag="agr", bufs=4
                    )
                    # quarter-granularity loads: first o_proj matmuls start as
                    # soon as the first 4 contraction chunks land
                    for q in range(2):
                        nc.sync.dma_start(
                            t[:, 4 * q : 4 * (q + 1), :],
                            agv[:, hf * (NK // 2) + 4 * q : hf * (NK // 2) + 4 * (q + 1), :],
                        )
                    agr.append(t)
                return agr

            agrs = {}

            def prefetch_oproj(b, n):
                # issue the agr DMAs well before the gen's MMs can reach the
                # head of the PE queue
                agrs[(b, n)] = load_agr(b, n)

            def push_oproj(b, n):
                agr = agrs.pop((b, n), None) or load_agr(b, n)
                g = gen_oproj(b, n, agr)
                gens[("oproj", b, n)] = g
                fillers.append(g)

            def emit_attn_stream(chunks):
                """All attention iterations of every (chunk, head-pair) as ONE
                flat stream with a 1-unit scores lookahead that crosses pair
                and chunk boundaries, so the ACT exp pipeline never restarts.

                chunks entries: dict(b, n, rate=1, pre=[...], acts={(p,j):
                [...]}, split_ag=False). `pre` callables run right before the
                chunk's first scores emission (emission-order safety for
                deps); `acts` run at the given (p, j) iteration.
                """
                seq = []
                for ci, ch in enumerate(chunks):
                    ch["jmax"] = (4 * ch["n"] + 4) if mode == "causal" else NK
                    for p in range(2):
                        for j in range(ch["jmax"]):
                            seq.append((ci, p, j))
                pS_tiles = {}
                pO = {}
                att = {}

                def emit_scores(u):
                    ci, p, j = u
                    ch = chunks[ci]
                    if p == 0 and j == 0:
                        for f in ch.get("pre", ()):
                            f()
                    b, n = ch["b"], ch["n"]
                    qT, sk = bt[b]["qT"], bt[b]["sk"]
                    nsl = slice(n * SC, (n + 1) * SC)
                    ksl = slice(j * 128, (j + 1) * 128)
                    pS = psS.tile([128, 2, SC], F32, name="pS", tag="psS")
                    nc.tensor.matmul(
                        pS[:, 0, :], sk[0:64, ksl], qT[p][0:64, nsl],
                        start=True, stop=True,
                    )
                    nc.tensor.matmul(
                        pS[:, 1, :], sk[64:128, ksl], qT[p][64:128, nsl],
                        start=True, stop=True,
                    )
                    pS_tiles[u] = pS

                def emit_exp_av(u):
                    ci, p, j = u
                    ch = chunks[ci]
                    b, n, jmax = ch["b"], ch["n"], ch["jmax"]
                    vnat = bt[b]["vnat"]
                    nsl = slice(n * SC, (n + 1) * SC)
                    pS = pS_tiles.pop(u)
                    eS = epool.tile([128, 2, SC], MMDT, name="eS", tag="eS")
                    if mode == "causal" and j >= 4 * n:
                        r = j - 4 * n
                        nc.scalar.activation(eS[:], pS[:], Exp, scale=0.125)
                        if r:
                            nc.vector.memset(eS[:, :, 0 : 128 * r], 0.0)
                        msl = slice(128 * r, 128 * (r + 1))
                        nc.vector.tensor_mul(
                            eS[:, :, msl], eS[:, :, msl], binm_sb[:]
                        )
                    elif mode == "general":
                        mt = mpool.tile([128, SC], F32, name="mt", tag="mt")
                        nc.sync.dma_start(
                            mt[:], maskT[128 * j : 128 * (j + 1), nsl]
                        )
                        nc.vector.scalar_tensor_tensor(
                            pS[:, 0, :], pS[:, 0, :], 0.125, mt[:], op0=Mult, op1=Add
                        )
                        nc.vector.scalar_tensor_tensor(
                            pS[:, 1, :], pS[:, 1, :], 0.125, mt[:], op0=Mult, op1=Add
                        )
                        nc.scalar.activation(eS[:], pS[:], Exp, scale=1.0)
                    else:
                        nc.scalar.activation(eS[:], pS[:], Exp, scale=0.125)
                    if j == 0:
                        pO[(ci, p)] = psV.tile([65, 2, SC], F32, name="pO", tag="psV")
                    po = pO[(ci, p)]
                    vsl = slice(j * 65, (j + 1) * 65)
                    nc.tensor.matmul(
                        po[:, 0, :], vnat[:, vsl], eS[:, 0, :],
                        start=(j == 0), stop=(j == jmax - 1),
                    )
                    nc.tensor.matmul(
                        po[:, 1, :], vnat[:, vsl], eS[:, 1, :],
                        start=(j == 0), stop=(j == jmax - 1),
                    )

                def emit_tail(u):
                    ci, p, j = u
                    ch = chunks[ci]
                    if j != ch["jmax"] - 1:
                        return
                    b, n = ch["b"], ch["n"]
                    po = pO.pop((ci, p))
                    last_unit = ci == len(chunks) - 1 and p == 1
                    pOc = npool.tile([65, 2, SC], F32, name="pOc", tag="pOc")
                    den0 = npool.tile([1, 2, SC], F32, name="den0", tag="den0", bufs=1)
                    if last_unit:
                        # tail-critical: pull the denominator row out first so
                        # recip + broadcast overlap the bulk PSUM drain
                        nc.vector.tensor_copy(den0[:], po[64:65, :, :])
                        nc.vector.reciprocal_approx_fast(den0[:], den0[:])
                        nc.vector.tensor_copy(pOc[:], po[:])
                        drive(1)
                    else:
                        # drain psum fast (frees the bank), then normalize
                        nc.vector.tensor_copy(pOc[:], po[:])
                        drive(1)
                        nc.sync.dma_start(den0[:], pOc[64:65, :, :])
                        nc.vector.reciprocal_approx_fast(den0[:], den0[:])
                    rb = npool.tile([64, 2, SC], F32, name="rb", tag="rb", bufs=1)
                    nc.gpsimd.partition_broadcast(rb[:], den0[:])
                    aT = atpool.tile([128, SC], MMDT, name=f"aT{p}", tag=f"aT{p}")
                    nc.vector.tensor_mul(aT[0:64, :], pOc[0:64, 0, :], rb[:, 0, :])
                    tb = npool.tile([64, SC], MMDT, name="tb", tag="tb", bufs=2)
                    nc.vector.tensor_mul(tb[:], pOc[0:64, 1, :], rb[:, 1, :])
                    nc.sync.dma_start(aT[64:128, :], tb[:])
                    att[(ci, p)] = aT
                    if ch.get("split_ag"):
                        # per-head-pair AllGather: p0's gather overlaps p1's
                        # attention so only p1's half is exposed in the tail
                        agi = dpool.tile(
                            [128, SC], MMDT, name=f"agis{b}_{n}_{p}",
                            tag="ag_in", bufs=4,
                        )
                        nc.sync.dma_start(agi[:], aT[:])
                        nc.gpsimd.collective_compute(
                            "AllGather",
                            mybir.AluOpType.bypass,
                            replica_groups=[list(range(NC))],
                            ins=[agi.opt()],
                            outs=[ag_out_sp[(b, n, p)].opt()],
                        )
                        drive(1)
                    elif p == 1:
                        ag_in = dpool.tile(
                            [QD, SC], MMDT, name=f"ag_in{b}_{n}", tag="ag_in", bufs=4
                        )
                        for pp in range(2):
                            nc.sync.dma_start(
                                ag_in[128 * pp : 128 * (pp + 1), :], att[(ci, pp)][:]
                            )
                        nc.gpsimd.collective_compute(
                            "AllGather",
                            mybir.AluOpType.bypass,
                            replica_groups=[list(range(NC))],
                            ins=[ag_in.opt()],
                            outs=[ag_out[(b, n)].opt()],
                        )
                        drive(2)

                emit_scores(seq[0])
                for i, u in enumerate(seq):
                    ci, p, j = u
                    ch = chunks[ci]
                    for f in ch.get("acts", {}).get((p, j), ()):
                        f()
                    if i + 1 < len(seq):
                        emit_scores(seq[i + 1])
                    if j % ch.get("rate", 1) == 0:
                        drive(1)
                    emit_exp_av(u)
                    emit_tail(u)

            def load_agr_split(b, n, p01):
                agv = ag_out_sp[(b, n, p01)].rearrange("(c r) s -> r c s", r=128)
                t = mpool.tile([128, NK // 2, SC], MMDT, name="agrs", tag="agr", bufs=4)
                for q in range(2):
                    nc.sync.dma_start(
                        t[:, 4 * q : 4 * (q + 1), :], agv[:, 4 * q : 4 * (q + 1), :]
                    )
                return t

            def emit_oproj_split(b, n):
                # o_proj consuming the two per-head-pair AllGathers: the p0
                # contraction chunks run while AG(p1) is still in flight
                nsl = slice(n * SC, (n + 1) * SC)
                ow_v = ow_sb.rearrange("p (c two) q -> p two c q", two=2)
                agr0 = agrs.pop((b, n, 0), None) or load_agr_split(b, n, 0)
                agr1 = load_agr_split(b, n, 1)
                ps = []
                for mt_i in range(2):
                    ps.append(psP.tile([128, SC], F32, name="ps_o", tag="psP"))
                    for c in range(8):
                        nc.tensor.matmul(
                            ps[mt_i][:],
                            ow_v[:, 0, c, 128 * mt_i : 128 * (mt_i + 1)],
                            agr0[:, c, :],
                            start=(c == 0), stop=False,
                        )
                for mt_i in range(2):
                    for c in range(8):
                        nc.tensor.matmul(
                            ps[mt_i][:],
                            ow_v[:, 1, c, 128 * mt_i : 128 * (mt_i + 1)],
                            agr1[:, c, :],
                            start=False, stop=(c == 7),
                        )
                    st = opool.tile([128, SC], F32, name="st")
                    nc.vector.tensor_copy(st[:], ps[mt_i][:])
                    nc.sync.dma_start(
                        y[b * QD + 128 * mt_i : b * QD + 128 * (mt_i + 1), nsl],
                        st[:],
                    )

            # ---- schedule ----
            # lead-in: proj(0,0) and proj(0,1) run dense on the PE while the
            # warm-up AllGather absorbs the collective start-up costs; the
            # remaining projections and all o_proj chunks ride the filler
            # queue between attention iterations.
            alloc_batch(0)
            alloc_batch(1)
            run_now(gen_proj(0, 0, xh00))
            run_now(gen_proj(0, 1, xh01))
            push_proj(0, 2)
            push_proj(0, 3)
            # in causal mode attention chunk (b,n) only reads keys j<=4n+3,
            # so later projections can ride the filler queue; in zeros/general
            # mode EVERY chunk reads the full key range, so all of a batch's
            # kv projections must be fully emitted before its first chunk
            causal = mode == "causal"
            pre00 = [] if causal else [
                lambda: force_done(("proj", 0, 2)),
                lambda: force_done(("proj", 0, 3)),
            ]
            pre10 = [lambda: force_done(("proj", 1, 0))] if causal else [
                lambda: force_done(("proj", 1, 0)),
                lambda: force_done(("proj", 1, 1)),
                lambda: force_done(("proj", 1, 2)),
                lambda: force_done(("proj", 1, 3)),
            ]
            chunks = [
                dict(b=0, n=0, pre=pre00),
                dict(b=0, n=1, pre=[lambda: push_proj(1, 0)]),
                dict(b=0, n=2, pre=[lambda: push_proj(1, 1)]),
                dict(b=0, n=3, pre=[lambda: push_proj(1, 2),
                                    lambda: push_proj(1, 3),
                                    lambda: push_oproj(0, 0)]),
                dict(b=1, n=0,
                     pre=pre10 + [lambda: push_oproj(0, 1)]),
                dict(b=1, n=1,
                     pre=([lambda: force_done(("proj", 1, 1))] if causal else [])
                         + [lambda: push_oproj(0, 2)]),
                dict(b=1, n=2,
                     pre=([lambda: force_done(("proj", 1, 2))] if causal else [])
                         + [lambda: push_oproj(0, 3)]),
                dict(b=1, n=3, split_ag=True,
                     pre=([lambda: force_done(("proj", 1, 3))] if causal else [])
                         + [lambda: push_oproj(1, 0)],
                     acts={(0, 0): [lambda: push_oproj(1, 1)],
                           (0, 8): [lambda: push_oproj(1, 2)],
                           (1, 2): [lambda: agrs.__setitem__(
                               (1, 3, 0), load_agr_split(1, 3, 0))]}),
            ]
            emit_attn_stream(chunks)
            # drain whatever filler is left, then the tail o_proj
            while fillers:
                drive(1)
            emit_oproj_split(1, 3)
    nc.compile()
    return nc


_cache = {}


def _get_nc(mode, shared_rope):
    key = (mode, shared_rope)
    if key not in _cache:
        _cache[key] = build(mode, shared_rope)
    return _cache[key]


def _mode_of(mask):
    m = np.asarray(mask)
    if not np.any(m):
        return "zeros"
    m2 = m.reshape(S, S)
    tril = np.tril(np.ones((S, S), dtype=bool))
    if np.all(m2[tril] == 0.0) and np.all(m2[~tril] <= -1e30):
        return "causal"
    return "general"


def make_inputs(hidden_states, cos, sin, positions, mask, q_w, q_b, k_w, k_b,
                v_w, v_b, o_w, mode):
    """Host-side preprocessing -> list of per-core input dicts."""
    ddt = ml_dtypes.bfloat16
    hs = np.ascontiguousarray(np.asarray(hidden_states, dtype=np.float32))
    xT = np.ascontiguousarray(hs.transpose(0, 2, 1).astype(ddt))   # [B, H, S]
    cos = np.asarray(cos, dtype=np.float32)
    sin = np.asarray(sin, dtype=np.float32)
    pos = np.asarray(positions)
    cosg = cos[pos]                                            # [B, S, 32]
    sing = sin[pos]
    d = np.arange(64)
    idx = d % 32
    sign = np.where(d % 2 == 0, -1.0, 1.0).astype(np.float32)
    C64 = cosg[:, :, idx].transpose(0, 2, 1)                   # [B, 64, S]
    Sn64 = (sing[:, :, idx] * sign[None, None, :]).transpose(0, 2, 1)
    ropeC = np.ascontiguousarray(np.concatenate([C64, C64], axis=1))   # [B,128,S]
    ropeS = np.ascontiguousarray(np.concatenate([Sn64, Sn64], axis=1))
    ident = np.eye(64, dtype=np.float32)

    q_w = np.asarray(q_w, dtype=np.float32)
    k_w = np.asarray(k_w, dtype=np.float32)
    v_w = np.asarray(v_w, dtype=np.float32)
    o_w = np.asarray(o_w, dtype=np.float32)
    q_b = np.asarray(q_b, dtype=np.float32)
    k_b = np.asarray(k_b, dtype=np.float32)
    v_b = np.asarray(v_b, dtype=np.float32)

    extra = {}
    if mode == "causal":
        kk = np.arange(128)[:, None]
        qq = np.arange(128)[None, :]
        extra["binm"] = np.ascontiguousarray(
            np.where(kk <= qq, 1.0, 0.0).astype(ddt)
        )
    if mode == "general":
        extra["maskT"] = np.ascontiguousarray(
            np.asarray(mask, dtype=np.float32).reshape(S, S).T
        )

    in_maps = []
    for c in range(NC):
        qsl = slice(c * QD, (c + 1) * QD)
        ksl = slice(c * HD, (c + 1) * HD)
        qbc = q_b[qsl]
        kvb_c = np.concatenate([k_b[ksl], v_b[ksl]])
        # pack weights into the exact SBUF tile layouts (contiguous DMA)
        qwt = q_w[qsl].T.reshape(NK, 128, QD)           # [k, p, q]
        qwP = np.ascontiguousarray(
            np.stack([
                np.stack([
                    qwt[hf * 8 : (hf + 1) * 8, :, 128 * m : 128 * (m + 1)]
                    .transpose(1, 0, 2) for hf in range(2)
                ]) for m in range(2)
            ]).astype(ddt)
        )                                               # [2, 2, 128, 8, 128]
        kvw = np.concatenate([k_w[ksl], v_w[ksl]], axis=0).T.reshape(NK, 128, 128)
        kvwP = np.ascontiguousarray(kvw.transpose(1, 0, 2).astype(ddt))
        ow = o_w[qsl, :].T.reshape(NK, 128, QD)
        owP = np.ascontiguousarray(ow.transpose(1, 0, 2).astype(ddt))
        m = {
            "xT": xT,
            "qwP": qwP,
            "kvwP": kvwP,
            "owP": owP,
            "qb": np.ascontiguousarray(qbc[:, None]),
            "qbs": np.ascontiguousarray(qbc[np.arange(QD) ^ 1][:, None]),
            "kvb": np.ascontiguousarray(kvb_c[:, None]),
            "kvbs": np.ascontiguousarray(kvb_c[np.arange(128) ^ 1][:, None]),
            "ropeC": ropeC,
            "ropeS": ropeS,
            "ident": ident,
        }
        m.update(extra)
        in_maps.append(m)
    return in_maps


def assemble_output(shards):
    """shards: list of per-core y arrays [B*QD, S] -> [B, S, H] float32."""
    full = np.empty((B, H, S), dtype=np.float32)
    for c in range(NC):
        sh = shards[c].reshape(B, QD, S)
        for b in range(B):
            full[b, QD * c : QD * (c + 1)] = sh[b]
    return np.ascontiguousarray(full.transpose(0, 2, 1).astype(np.float32))


def kernel(**inputs):
    mode = _mode_of(inputs["mask"])
    pos = np.asarray(inputs["positions"])
    shared_rope = bool(np.array_equal(pos[0], pos[1]))
    nc = _get_nc(mode, shared_rope)
    in_maps = make_inputs(mode=mode, **{k: inputs[k] for k in (
        "hidden_states", "cos", "sin", "positions", "mask",
        "q_w", "q_b", "k_w", "k_b", "v_w", "v_b", "o_w")})
    res = run_bass_kernel_spmd(nc, in_maps, list(range(NC)))
    return assemble_output([res.results[c]["y"] for c in range(NC)])
